# revision 12
# baseline (speedup 1.0000x reference)
"""Multi-head attention (B=8, N=1024, D=768, H=12) on 8 TRN2 NeuronCores.

Sharding: data-parallel over batch B - one batch element per core, weights
replicated, no collectives.

Design (v5 = v2 + faster prologue): the kernel is a two-engine race
between the PE (matmuls, ~115us) and ACT (96 exps of [128,1024], ~110us).
Everything else hides.  v5 changes vs v2: (1) input DMA batched into 9
large transfers across 3 queues, with the 4.4MB of weights the first exp
does NOT need dep-chained behind x qb1 so they don't steal HBM bandwidth
from the exp-critical 1.9MB; (2) 14 PE warmup matmuls spanning the DMA
window so HAM holds K=8/8 into the first QK matmuls.

Per-core layout (feature-major, no on-chip transposes):
  x^T [768, 1024] (host-transposed, bf16)
  Q/K feature-major [c, n]: lhsT = w_qkv block, rhs = x^T          -> QK_fm
  V token-major  [n, c]:    lhsT = x^T block,  rhs = w_qkv V cols  -> V_tm
     (plain [128, 768] tiles - no ones column)
  S^T[k, q] per (pair, j, ktile): lhsT = K_fm [64,128], rhs = Q_fm [64,512]
     row-tiled j0/j1 (partitions 0-63 / 64-127) for PE-array overlap
  P^T = exp(SCALE * S^T) on ACT, bf16, one [128,1024] instr per (j, kt)
  AV: column-tiled pure-V matmuls, M=64 per head: j0 -> psum rows 0-63,
     j1 -> rows 64-127 (tile_position (0,0)/(0,64)), accumulated over kt
     into av_ps[qb] [128,512].  Softmax denominators via a separate
     4x-column-tiled ones-matmul pass: lhsT = ones[128,1], out rows
     0/32/64/96 of one psum bank = (j0,qb0),(j1,qb0),(j0,qb1),(j1,qb1).
     Shared-bank accumulation groups are safe: start=True clears
     has_written only for the matmul's own output region (measured), so
     each region's first MM carries start=True.
  norm: recip of the 4 denominator rows (strided-partition DVE), DRAM
     round-trip broadcast to rbc [128,1024] (pairs 0-4), then one
     tensor_mul(av_psum, rbc_sbuf) -> at per qb.  Pair 5 uses the same
     DRAM path issued at body-5 step 1.
  proj: lhsT = w_proj block, rhs = at -> out_fm [768,1024] bf16 + bias
     (bias folded into the DVE/ACT eviction).  mb0/1 prefill c0-c4 in ST
     slots during pair 5, mb2qb0 in the den slot; mb3-5 run in the tail.
Host gathers out_fm (bf16), upcasts, transposes back to [B, 1024, 768].

PSUM budget (8 banks): st 2x[128,1024]=4, av 2x[128,512]=2, den 1, sh 1.
In the prologue the av ring doubles as QK/V eviction staging; in the
pair loop QK(p+2) uses the single sh slot.  PE warmup matmuls run during
the initial DMA window so HAM reaches K=8/8 before the first real MM.
"""

import numpy as np
import ml_dtypes

import concourse.bass as bass
import concourse.tile as tile
from concourse import bacc, mybir
from concourse.tile import add_dep_helper

FP32 = mybir.dt.float32
BF16 = mybir.dt.bfloat16

B, N, D = 8, 1024, 768
H, HD = 12, 64
SCALE = float(HD) ** -0.5  # 0.125
CB = D // 128  # 6 contraction blocks of 128
PAIRS = H // 2  # 6 head pairs
KT = N // 128  # 8 key-token tiles
QB = N // 512  # 2 q blocks of 512
NCORES = 8
W3 = 3 * D  # 2304


def build_attention(tc, outs, ins):
    from contextlib import ExitStack

    nc = tc.nc
    xT = ins["xT"]  # [768, 1024] bf16 dram
    wqkv = ins["w_qkv"]  # [768, 2304] bf16 dram
    wproj = ins["w_proj"]  # [768, 768] bf16 dram
    bproj = ins["b_proj"]  # [768] fp32 dram
    out = outs["out"]  # [768, 1024] bf16 dram

    Exp = mybir.ActivationFunctionType.Exp

    with ExitStack() as ctx:
        ec = ctx.enter_context
        sb_x = ec(tc.tile_pool(name="sb_x", bufs=1))
        sb_wqk0 = ec(tc.tile_pool(name="sb_wqk0", bufs=1))
        sb_wqkr = ec(tc.tile_pool(name="sb_wqkr", bufs=1))
        sb_wv = ec(tc.tile_pool(name="sb_wv", bufs=1))
        sb_wproj = ec(tc.tile_pool(name="sb_wproj", bufs=1))
        sb_misc = ec(tc.tile_pool(name="sb_misc", bufs=1))
        sb_qk = ec(tc.tile_pool(name="sb_qk", bufs=6))
        sb_v = ec(tc.tile_pool(name="sb_v", bufs=KT))
        sb_pt = ec(tc.tile_pool(name="sb_pt", bufs=40))
        sb_den = ec(tc.tile_pool(name="sb_den", bufs=2))
        sb_rbc = ec(tc.tile_pool(name="sb_rbc", bufs=2))
        sb_at = ec(tc.tile_pool(name="sb_at", bufs=PAIRS))
        sb_stage = ec(tc.tile_pool(name="sb_stage", bufs=2))
        sb_out = ec(tc.tile_pool(name="sb_out", bufs=3))
        ps_st = ec(tc.tile_pool(name="ps_st", bufs=2, space="PSUM"))
        ps_av = ec(tc.tile_pool(name="ps_av", bufs=2, space="PSUM"))
        ps_den = ec(tc.tile_pool(name="ps_den", bufs=1, space="PSUM"))
        ps_sh = ec(tc.tile_pool(name="ps_sh", bufs=1, space="PSUM"))
        dram = ec(tc.tile_pool(name="dram", bufs=1, space="DRAM"))

        # ---------- constants / warmup ----------
        warm = sb_misc.tile([128, 512], BF16, name="warm", tag="warm")
        nc.vector.memset(warm, 0.0)
        ones_col = sb_misc.tile([128, 1], BF16, name="ones_col", tag="onesc")
        nc.vector.memset(ones_col, 1.0)
        ones_row = sb_misc.tile([128, 64], BF16, name="ones_row", tag="onesr")
        nc.vector.memset(ones_row, 1.0)
        bias_sb = sb_misc.tile([128, CB], FP32, name="bias")

        # trigger the ACT exp table load (~2.7us) during the initial DMA
        # window instead of serializing it before the first real exp
        nc.scalar.activation(warm[0:1, 0:16], warm[0:1, 16:32], Exp, scale=1.0)

        # PE warmup: ~14 N=512 matmuls spanning the input-DMA window keep
        # the PE busy so HAM un-throttles to K=8/8 before the first real
        # matmul (which otherwise runs the whole first QK block at 1.2GHz)
        warm_ps = ps_sh.tile([128, 512], FP32, name="warm_ps", tag="sh")
        for i in range(14):
            nc.tensor.matmul(
                warm_ps, lhsT=warm[:, 0:128], rhs=warm, start=True, stop=True
            )

        # ---------- batched input loads, 3 queues, need-ordered ----------
        x_all = sb_x.tile([128, CB * N], BF16, name="x_all", tag="x")
        d_xqb1 = nc.sync.dma_start(
            x_all.rearrange("p (c n) -> p c n", c=CB),
            bass.AP(tensor=xT.tensor, offset=xT.offset, ap=[[N, 128], [128 * N, CB], [1, N]]),
        )
        # pair-0 Q/K weights as 12 small per-c DMAs: each completes fast,
        # so the 256B-line penalty never gates the first QK matmuls
        wqk0_t = sb_wqk0.tile([128, CB * 256], BF16, name="wqk0", tag="wqk0")
        for c in range(CB):
            for w in range(2):
                nc.scalar.dma_start(
                    wqk0_t[:, c * 256 + w * 128 : c * 256 + (w + 1) * 128],
                    bass.AP(
                        tensor=wqkv.tensor,
                        offset=wqkv.offset + c * 128 * W3 + w * D,
                        ap=[[W3, 128], [1, 128]],
                    ),
                )
        # the 4.4MB of weights the first exp does NOT need: chained after
        # x qb1 so they don't steal HBM bandwidth from the critical path
        wv_all = sb_wv.tile([128, CB * D], BF16, name="wv", tag="wv")
        d_wv = nc.scalar.dma_start(
            wv_all.rearrange("p (c n) -> p c n", c=CB),
            bass.AP(tensor=wqkv.tensor, offset=wqkv.offset + 2 * D, ap=[[W3, 128], [128 * W3, CB], [1, D]]),
        )
        add_dep_helper(d_wv.ins, d_xqb1.ins, sync=True, reason="bw order wv")
        wqkr_t = sb_wqkr.tile([128, CB * 1280], BF16, name="wqkr", tag="wqkr")
        for w in range(2):
            d_wqkr = nc.gpsimd.dma_start(
                wqkr_t.rearrange("p (c w m) -> p c w m", c=CB, w=2)[:, :, w],
                bass.AP(tensor=wqkv.tensor, offset=wqkv.offset + 128 + w * D, ap=[[W3, 128], [128 * W3, CB], [1, 640]]),
            )
            add_dep_helper(d_wqkr.ins, d_xqb1.ins, sync=True, reason="bw order wqkr")
        wp_all = sb_wproj.tile([128, CB * D], BF16, name="wp", tag="wp")
        nc.gpsimd.dma_start(
            wp_all.rearrange("p (c n) -> p c n", c=CB),
            bass.AP(tensor=wproj.tensor, offset=wproj.offset, ap=[[D, 128], [128 * D, CB], [1, D]]),
        )
        nc.gpsimd.dma_start(bias_sb, bproj.rearrange("(a p) -> p a", p=128))
        s_dram = dram.tile([PAIRS, 4, 512], FP32, name="s_dram")

        def x_sl(c, lo, hi):
            return x_all[:, c * N + lo : c * N + hi]

        def wqk_slice(c, p, which):
            if p == 0:
                return wqk0_t[:, c * 256 + which * 128 : c * 256 + (which + 1) * 128]
            o = c * 1280 + which * 640 + (p - 1) * 128
            return wqkr_t[:, o : o + 128]

        def wp_slice(c, mb):
            return wp_all[:, c * D + mb * 128 : c * D + (mb + 1) * 128]

        # ---------- QK projection groups (prologue ring: sh + av slots) ----
        qk_sb = {}  # (which, pair) -> [128, N] bf16
        _ring_state = [0]

        def ring_tile(name):
            # round-robin [sh, av, av] staging slots for QK/V psum groups
            i = _ring_state[0] % 3
            _ring_state[0] += 1
            if i == 0:
                return ps_sh.tile([128, 512], FP32, name=name, tag="sh")
            return ps_av.tile([128, 512], FP32, name=name, tag="av")

        def emit_qk_group(p, which, qb, body=False, slot=None):
            if (which, p) not in qk_sb:
                qkt = sb_qk.tile([128, N], BF16, name=f"qk{which}_{p}", tag="qk")
                qk_sb[(which, p)] = qkt
            qkt = qk_sb[(which, p)]
            if slot == "den":
                ps = ps_den.tile([128, 512], FP32, name=f"qkps{which}_{p}_{qb}", tag="den")
            elif body:
                ps = ps_sh.tile([128, 512], FP32, name=f"qkps{which}_{p}_{qb}", tag="sh")
            else:
                ps = ring_tile(f"qkps{which}_{p}_{qb}")
            for c in range(CB):
                nc.tensor.matmul(
                    ps,
                    lhsT=wqk_slice(c, p, which),
                    rhs=x_sl(c, qb * 512, (qb + 1) * 512),
                    start=(c == 0),
                    stop=(c == CB - 1),
                )
            nc.vector.tensor_copy(qkt[:, qb * 512 : (qb + 1) * 512], ps)

        # ---------- V projection ----------
        v_sb = []

        def emit_v(t, sh_only=False):
            vt = sb_v.tile([128, D], BF16, name=f"v{t}", tag="v")
            for n0, nw in ((0, 512), (512, 256)):
                if sh_only:
                    vps = ps_sh.tile([128, 512], FP32, name=f"vps{t}_{n0}", tag="sh")
                else:
                    vps = ring_tile(f"vps{t}_{n0}")
                for c in range(CB):
                    nc.tensor.matmul(
                        vps[:, 0:nw],
                        lhsT=x_sl(c, t * 128, (t + 1) * 128),
                        rhs=wv_all[:, c * D + n0 : c * D + n0 + nw],
                        start=(c == 0),
                        stop=(c == CB - 1),
                    )
                nc.vector.tensor_copy(vt[:, n0 : n0 + nw], vps[:, 0:nw])
            v_sb.append(vt)

        # ---------- S^T + exp ----------
        pt_tiles = {}  # (pair, j, kt) -> [128, N] bf16

        def st_src(st):
            # one [128, N] view across the tile's two psum banks
            return bass.AP(
                tensor=st.tensor, offset=st.offset, ap=[st.ap[0], [1, N]]
            )

        def emit_st_exp(p, kt):
            # Both heads' S^T for this ktile with alternating row groups
            # (partitions 0-63 / 64-127) so consecutive matmuls overlap in
            # the PE array, then one [128,1024] exp per head on ACT.
            q_t, k_t = qk_sb[(0, p)], qk_sb[(1, p)]
            sts = []
            for j in (0, 1):
                st = ps_st.tile([128, N], FP32, name=f"st{2*p+j}_{kt}", tag="st")
                sts.append(st)
            # j-major order: both qb halves of j0 first (same row group,
            # same lhsT -> back-to-back), then j1.  Keeps exp(j0) off j1's
            # slot-wait; measured faster than qb-major co-issue attempts.
            prev_mm = None
            for j in (0, 1):
                for qb in range(QB):
                    mm = nc.tensor.matmul(
                        sts[j][:, qb * 512 : (qb + 1) * 512],
                        lhsT=k_t[j * 64 : (j + 1) * 64, kt * 128 : (kt + 1) * 128],
                        rhs=q_t[j * 64 : (j + 1) * 64, qb * 512 : (qb + 1) * 512],
                        start=True,
                        stop=True,
                    )
                    if prev_mm is not None:
                        add_dep_helper(
                            mm.ins,
                            prev_mm.ins,
                            sync=False,
                            reason="pin ST issue order",
                        )
                    prev_mm = mm
                pt = sb_pt.tile([128, N], BF16, name=f"pt{2*p+j}_{kt}", tag="pt")
                nc.scalar.activation(pt, st_src(sts[j]), Exp, scale=SCALE)
                pt_tiles[(p, j, kt)] = pt

        # ---------- denominators ----------
        den_ps = {}  # pair -> [128, 512] psum (rows 0/32/64/96 valid)

        def emit_den_kt(p, kt):
            # 4x-column-tiled ones-matmuls: all four (j, qb) denominators
            # of this ktile accumulate concurrently in one psum bank at
            # partition rows 0/32/64/96.
            if p not in den_ps:
                den_ps[p] = ps_den.tile([128, 512], FP32, name=f"den{p}", tag="den")
                if p == 0:
                    # one-time init of the never-written garbage rows so
                    # the later full-tile reciprocal reads defined data
                    nc.vector.memset(den_ps[p], 1.0)
            dps = den_ps[p]
            for j in (0, 1):
                for qb in range(QB):
                    r = 32 * (2 * qb + j)
                    nc.tensor.matmul(
                        dps[r : r + 1, :],
                        lhsT=ones_col,
                        rhs=pt_tiles[(p, j, kt)][:, qb * 512 : (qb + 1) * 512],
                        start=(kt == 0),
                        stop=(kt == KT - 1),
                        skip_group_check=True,
                        tile_position=(0, r),
                    )

        # ---------- AV (column-tiled, M=64 per head) ----------
        def emit_av_q(p, tile_, qb, kt):
            for j in (0, 1):
                h = 2 * p + j
                nc.tensor.matmul(
                    tile_[j * 64 : (j + 1) * 64, :],
                    lhsT=v_sb[kt][:, h * 64 : (h + 1) * 64],
                    rhs=pt_tiles[(p, j, kt)][:, qb * 512 : (qb + 1) * 512],
                    start=(kt == 0),
                    stop=(kt == KT - 1),
                    skip_group_check=True,
                )

        def emit_av_kt(p, av_tiles, kt, only_qb=None):
            # j-major: both qb halves of a head share the stationary V
            # tile (one LDWEIGHTS); j1's column group overlaps j0's tail
            for j in (0, 1):
                h = 2 * p + j
                for qb in range(QB):
                    if only_qb is not None and qb != only_qb:
                        continue
                    nc.tensor.matmul(
                        av_tiles[qb][j * 64 : (j + 1) * 64, :],
                        lhsT=v_sb[kt][:, h * 64 : (h + 1) * 64],
                        rhs=pt_tiles[(p, j, kt)][:, qb * 512 : (qb + 1) * 512],
                        start=(kt == 0),
                        stop=(kt == KT - 1),
                        skip_group_check=True,
                    )

        # ---------- normalization ----------
        def emit_recip(p):
            # reciprocal straight from the den psum bank (rows 0/32/64/96
            # valid; garbage rows were memset to 1.0 once in the prologue).
            dps = den_ps[p]
            rsb = sb_den.tile([128, 512], FP32, name=f"rsb{p}", tag="rsb")
            nc.vector.reciprocal_approx_fast(rsb, dps)
            return rsb

        def emit_norm_bcast(p, rsb):
            # DRAM round-trip broadcast: rsb rows -> s_dram[p] -> rbc tile
            # [128, 1024] (rows 0-63 = 1/d(j0), rows 64-127 = 1/d(j1))
            sd = s_dram[p]
            for i in range(4):
                nc.sync.dma_start(sd[i], rsb[32 * i : 32 * i + 1, :])
            rbc = sb_rbc.tile([128, N], FP32, name=f"rbc{p}", tag="rbc")
            for j in (0, 1):
                row = sd[j]
                bcast = bass.AP(
                    tensor=row.tensor,
                    offset=row.offset,
                    ap=[[0, 64], [2 * 512, 2], [1, 512]],
                )
                nc.gpsimd.dma_start(
                    rbc[j * 64 : (j + 1) * 64, :].rearrange(
                        "p (a b) -> p a b", a=2
                    ),
                    bcast,
                )
            return rbc

        def emit_norm_mul(p, av_tiles, rbc, at):
            for qb in range(QB):
                nc.vector.tensor_mul(
                    at[:, qb * 512 : (qb + 1) * 512],
                    av_tiles[qb],
                    rbc[:, qb * 512 : (qb + 1) * 512],
                )

        # ---------- output projection ----------
        at_sb = []

        def emit_proj_k(ps_pair, mb, c_lo, c_hi):
            for qb in range(QB):
                for c in range(c_lo, c_hi):
                    nc.tensor.matmul(
                        ps_pair[qb],
                        lhsT=wp_slice(c, mb),
                        rhs=at_sb[c][:, qb * 512 : (qb + 1) * 512],
                        start=(c == 0),
                        stop=(c == CB - 1),
                    )

        def emit_proj_out(ps_pair, mb, on_act=True):
            # bias-add eviction + bf16 store; qb1 evicts on the (idle) ACT
            ot = sb_out.tile([128, N], BF16, name=f"out{mb}", tag="out")
            for qb in range(QB):
                dst = ot[:, qb * 512 : (qb + 1) * 512]
                if on_act and qb == 1:
                    nc.scalar.add(dst, ps_pair[qb], bias_sb[:, mb : mb + 1])
                else:
                    nc.vector.tensor_scalar_add(
                        dst, ps_pair[qb], bias_sb[:, mb : mb + 1]
                    )
                nc.sync.dma_start(
                    out[mb * 128 : (mb + 1) * 128, qb * 512 : (qb + 1) * 512],
                    dst,
                )

        # =======================================================
        # prologue: QK(0) 4-way, then per-kt [ST(0)+exp(0), den(0) lag-2,
        # V(0..5), QK(1)].  den(0) kt6/7, V(6/7) wrap into body 0.
        # =======================================================
        emit_qk_group(0, 0, 0)
        emit_qk_group(0, 1, 0)
        emit_qk_group(0, 0, 1)
        emit_qk_group(0, 1, 1, slot="den")
        for kt in range(KT):
            emit_st_exp(0, kt)
            if kt >= 2:
                emit_den_kt(0, kt - 2)
            if kt < 6:
                emit_v(kt)
            if kt >= 4:
                qb_, which_ = divmod(kt - 4, 2)
                emit_qk_group(1, which_, qb_)

        # =======================================================
        # pipelined pairs.  body(p) steady state per kt:
        #   [ST(p+1,kt), den(p+1,kt-2), AV(p,kt-2), QK(p+2)]
        # kt6/7 leftovers of den(p+1)/AV(p) WRAP into body(p+1) steps 0/1
        # so pair boundaries never burst (keeps the exp stream gapless).
        # norm(p-1) runs at body(p) step 1 once its wrapped AV lands.
        # AV(5,qb0) accumulates in the sh slot during body 4 (QK is done),
        # so body 5 only owes AV(5,qb1) + norm + proj.
        # =======================================================
        prev_av = None
        prev_rbc = None
        av5_q0 = None
        for p in range(PAIRS):
            last = p == PAIRS - 1
            at = sb_at.tile([128, N], BF16, name=f"attn{p}", tag="attn")
            at_sb.append(at)
            if last:
                av_tiles = [
                    av5_q0,
                    ps_av.tile([128, 512], FP32, name="av5_1", tag="av"),
                ]
                mb01_ps = []
            else:
                av_tiles = [
                    ps_av.tile([128, 512], FP32, name=f"av{p}_{qb}", tag="av")
                    for qb in range(QB)
                ]

            for kt in range(KT):
                if not last:
                    emit_st_exp(p + 1, kt)
                if p == 0 and kt < 2:
                    emit_v(KT - 2 + kt, sh_only=True)
                if kt < 2:
                    # wrapped leftovers from the previous body
                    emit_den_kt(p, KT - 2 + kt)
                    if p >= 1:
                        emit_av_kt(p - 1, prev_av, KT - 2 + kt)
                        if p == 5:
                            emit_av_q(5, av5_q0, 0, KT - 2 + kt)
                    if kt == 1:
                        rsb = emit_recip(p)
                        if p >= 1:
                            emit_norm_mul(p - 1, prev_av, prev_rbc, at_sb[p - 1])
                        rbc = emit_norm_bcast(p, rsb)
                else:
                    if not last:
                        emit_den_kt(p + 1, kt - 2)
                    # pair 5's qb0 was already accumulated during body 4
                    emit_av_kt(p, av_tiles, kt - 2, only_qb=1 if last else None)
                    if p == 4:
                        if kt == 2:
                            av5_q0 = ps_sh.tile(
                                [128, 512], FP32, name="av5_0", tag="sh"
                            )
                        emit_av_q(5, av5_q0, 0, kt - 2)
                if p + 2 < PAIRS and 2 <= kt <= 5:
                    qb_, which_ = divmod(kt - 2, 2)
                    emit_qk_group(p + 2, which_, qb_, body=True)
                if last and kt == 1:
                    t0 = ps_st.tile([128, N], FP32, name="projps0", tag="st")
                    mb01_ps.append([t0[:, 0:512], t0[:, 512:1024]])
                    emit_proj_k(mb01_ps[0], 0, 0, CB - 1)
                if last and kt == 2:
                    t1 = ps_st.tile([128, N], FP32, name="projps1", tag="st")
                    mb01_ps.append([t1[:, 0:512], t1[:, 512:1024]])
                    emit_proj_k(mb01_ps[1], 1, 0, CB - 1)
                if last and kt == 3:
                    mb2q0 = ps_den.tile(
                        [128, 512], FP32, name="projps2a", tag="den"
                    )
                    for c in range(CB - 1):
                        nc.tensor.matmul(
                            mb2q0,
                            lhsT=wp_slice(c, 2),
                            rhs=at_sb[c][:, 0:512],
                            start=(c == 0),
                            stop=False,
                        )
            if last:
                # finish AV(5,qb0) kt6/7 (sh) happened in wrap above; qb1:
                emit_av_kt(p, av_tiles, KT - 2, only_qb=1)
                emit_av_kt(p, av_tiles, KT - 1, only_qb=1)
                emit_norm_mul(p, av_tiles, rbc, at)
            else:
                prev_av = av_tiles
                prev_rbc = rbc

        # =======================================================
        # projection tail
        # =======================================================
        emit_proj_k(mb01_ps[0], 0, CB - 1, CB)
        emit_proj_out(mb01_ps[0], 0)
        emit_proj_k(mb01_ps[1], 1, CB - 1, CB)
        emit_proj_out(mb01_ps[1], 1)
        nc.tensor.matmul(
            mb2q0,
            lhsT=wp_slice(CB - 1, 2),
            rhs=at_sb[CB - 1][:, 0:512],
            start=False,
            stop=True,
        )
        mb2q1 = ps_av.tile([128, 512], FP32, name="projps2b", tag="av")
        for c in range(CB):
            nc.tensor.matmul(
                mb2q1,
                lhsT=wp_slice(c, 2),
                rhs=at_sb[c][:, 512:1024],
                start=(c == 0),
                stop=(c == CB - 1),
            )
        emit_proj_out([mb2q0, mb2q1], 2)
        t3 = ps_st.tile([128, N], FP32, name="projps3", tag="st")
        mb3_ps = [t3[:, 0:512], t3[:, 512:1024]]
        emit_proj_k(mb3_ps, 3, 0, CB)
        emit_proj_out(mb3_ps, 3)
        t4 = ps_st.tile([128, N], FP32, name="projps4", tag="st")
        mb4_ps = [t4[:, 0:512], t4[:, 512:1024]]
        emit_proj_k(mb4_ps, 4, 0, CB)
        emit_proj_out(mb4_ps, 4)
        mb5a = ps_sh.tile([128, 512], FP32, name="projps5a", tag="sh")
        mb5b = ps_den.tile([128, 512], FP32, name="projps5b", tag="den")
        mb5_ps = [mb5a, mb5b]
        emit_proj_k(mb5_ps, 5, 0, CB)
        emit_proj_out(mb5_ps, 5)


def build_nc():
    nc = bacc.Bacc(
        "TRN2", target_bir_lowering=False, debug=False, num_devices=NCORES
    )
    ins = {
        "xT": nc.dram_tensor("xT", [D, N], BF16, kind="ExternalInput").ap(),
        "w_qkv": nc.dram_tensor("w_qkv", [D, 3 * D], BF16, kind="ExternalInput").ap(),
        "w_proj": nc.dram_tensor("w_proj", [D, D], BF16, kind="ExternalInput").ap(),
        "b_proj": nc.dram_tensor("b_proj", [D], FP32, kind="ExternalInput").ap(),
    }
    outs = {"out": nc.dram_tensor("out", [D, N], BF16, kind="ExternalOutput").ap()}
    with tile.TileContext(nc) as tc:
        build_attention(tc, outs, ins)
    nc.compile()
    return nc


def make_in_maps(x, w_qkv, w_proj, b_proj):
    xT = np.ascontiguousarray(
        np.transpose(np.asarray(x, np.float32), (0, 2, 1))
    ).astype(ml_dtypes.bfloat16)
    wq = np.asarray(w_qkv, np.float32).astype(ml_dtypes.bfloat16)
    wp = np.asarray(w_proj, np.float32).astype(ml_dtypes.bfloat16)
    bp = np.ascontiguousarray(np.asarray(b_proj, np.float32))
    return [
        {"xT": np.ascontiguousarray(xT[b]), "w_qkv": wq, "w_proj": wp, "b_proj": bp}
        for b in range(B)
    ]


_BUILT = None


def _get_built():
    global _BUILT
    if _BUILT is None:
        _BUILT = build_nc()
    return _BUILT


def kernel(x, w_qkv, w_proj, b_proj):
    from concourse.bass_utils import run_bass_kernel_spmd

    nc = _get_built()
    in_maps = make_in_maps(x, w_qkv, w_proj, b_proj)
    res = run_bass_kernel_spmd(nc, in_maps, core_ids=list(range(NCORES)))
    return np.stack(
        [
            np.asarray(res.results[b]["out"], np.float32).T
            for b in range(B)
        ]
    )


# revision 13
# speedup vs baseline: 1.0197x; 1.0197x over previous
"""Multi-head attention (B=8, N=1024, D=768, H=12) on 8 TRN2 NeuronCores.

Sharding: data-parallel over batch B - one batch element per core, weights
replicated, no collectives.

Design (v5 = v2 + faster prologue): the kernel is a two-engine race
between the PE (matmuls, ~115us) and ACT (96 exps of [128,1024], ~110us).
Everything else hides.  v5 changes vs v2: (1) input DMA batched into 9
large transfers across 3 queues, with the 4.4MB of weights the first exp
does NOT need dep-chained behind x qb1 so they don't steal HBM bandwidth
from the exp-critical 1.9MB; (2) 14 PE warmup matmuls spanning the DMA
window so HAM holds K=8/8 into the first QK matmuls.

Per-core layout (feature-major, no on-chip transposes):
  x^T [768, 1024] (host-transposed, bf16)
  Q/K feature-major [c, n]: lhsT = w_qkv block, rhs = x^T          -> QK_fm
  V token-major  [n, c]:    lhsT = x^T block,  rhs = w_qkv V cols  -> V_tm
     (plain [128, 768] tiles - no ones column)
  S^T[k, q] per (pair, j, ktile): lhsT = K_fm [64,128], rhs = Q_fm [64,512]
     row-tiled j0/j1 (partitions 0-63 / 64-127) for PE-array overlap
  P^T = exp(SCALE * S^T) on ACT, bf16, one [128,1024] instr per (j, kt)
  AV: column-tiled pure-V matmuls, M=64 per head: j0 -> psum rows 0-63,
     j1 -> rows 64-127 (tile_position (0,0)/(0,64)), accumulated over kt
     into av_ps[qb] [128,512].  Softmax denominators via a separate
     4x-column-tiled ones-matmul pass: lhsT = ones[128,1], out rows
     0/32/64/96 of one psum bank = (j0,qb0),(j1,qb0),(j0,qb1),(j1,qb1).
     Shared-bank accumulation groups are safe: start=True clears
     has_written only for the matmul's own output region (measured), so
     each region's first MM carries start=True.
  norm: recip of the 4 denominator rows (strided-partition DVE), DRAM
     round-trip broadcast to rbc [128,1024] (pairs 0-4), then one
     tensor_mul(av_psum, rbc_sbuf) -> at per qb.  Pair 5 uses the same
     DRAM path issued at body-5 step 1.
  proj: lhsT = w_proj block, rhs = at -> out_fm [768,1024] bf16 + bias
     (bias folded into the DVE/ACT eviction).  mb0/1 prefill c0-c4 in ST
     slots during pair 5, mb2qb0 in the den slot; mb3-5 run in the tail.
Host gathers out_fm (bf16), upcasts, transposes back to [B, 1024, 768].

PSUM budget (8 banks): st 2x[128,1024]=4, av 2x[128,512]=2, den 1, sh 1.
In the prologue the av ring doubles as QK/V eviction staging; in the
pair loop QK(p+2) uses the single sh slot.  PE warmup matmuls run during
the initial DMA window so HAM reaches K=8/8 before the first real MM.
"""

import numpy as np
import ml_dtypes

import concourse.bass as bass
import concourse.tile as tile
from concourse import bacc, mybir
from concourse.tile import add_dep_helper

FP32 = mybir.dt.float32
BF16 = mybir.dt.bfloat16

B, N, D = 8, 1024, 768
H, HD = 12, 64
SCALE = float(HD) ** -0.5  # 0.125
CB = D // 128  # 6 contraction blocks of 128
PAIRS = H // 2  # 6 head pairs
KT = N // 128  # 8 key-token tiles
QB = N // 512  # 2 q blocks of 512
NCORES = 8
W3 = 3 * D  # 2304


def build_attention(tc, outs, ins):
    from contextlib import ExitStack

    nc = tc.nc
    xT = ins["xT"]  # [768, 1024] bf16 dram
    wqkv = ins["w_qkv"]  # [768, 2304] bf16 dram
    wproj = ins["w_proj"]  # [768, 768] bf16 dram
    bproj = ins["b_proj"]  # [768] fp32 dram
    out = outs["out"]  # [768, 1024] bf16 dram

    Exp = mybir.ActivationFunctionType.Exp

    with ExitStack() as ctx:
        ec = ctx.enter_context
        sb_x = ec(tc.tile_pool(name="sb_x", bufs=1))
        sb_wqk0 = ec(tc.tile_pool(name="sb_wqk0", bufs=1))
        sb_wqkr = ec(tc.tile_pool(name="sb_wqkr", bufs=1))
        sb_wv = ec(tc.tile_pool(name="sb_wv", bufs=1))
        sb_wproj = ec(tc.tile_pool(name="sb_wproj", bufs=1))
        sb_misc = ec(tc.tile_pool(name="sb_misc", bufs=1))
        sb_qk = ec(tc.tile_pool(name="sb_qk", bufs=6))
        sb_v = ec(tc.tile_pool(name="sb_v", bufs=KT))
        sb_pt = ec(tc.tile_pool(name="sb_pt", bufs=40))
        sb_den = ec(tc.tile_pool(name="sb_den", bufs=2))
        sb_rbc = ec(tc.tile_pool(name="sb_rbc", bufs=2))
        sb_at = ec(tc.tile_pool(name="sb_at", bufs=PAIRS))
        sb_stage = ec(tc.tile_pool(name="sb_stage", bufs=2))
        sb_out = ec(tc.tile_pool(name="sb_out", bufs=3))
        ps_st = ec(tc.tile_pool(name="ps_st", bufs=2, space="PSUM"))
        ps_av = ec(tc.tile_pool(name="ps_av", bufs=2, space="PSUM"))
        ps_den = ec(tc.tile_pool(name="ps_den", bufs=1, space="PSUM"))
        ps_sh = ec(tc.tile_pool(name="ps_sh", bufs=1, space="PSUM"))
        dram = ec(tc.tile_pool(name="dram", bufs=1, space="DRAM"))

        # ---------- constants / warmup ----------
        warm = sb_misc.tile([128, 512], BF16, name="warm", tag="warm")
        nc.vector.memset(warm, 0.0)
        ones_col = sb_misc.tile([128, 1], BF16, name="ones_col", tag="onesc")
        nc.vector.memset(ones_col, 1.0)
        ones_row = sb_misc.tile([128, 64], BF16, name="ones_row", tag="onesr")
        nc.vector.memset(ones_row, 1.0)
        bias_sb = sb_misc.tile([128, CB], FP32, name="bias")

        # trigger the ACT exp table load (~2.7us) during the initial DMA
        # window instead of serializing it before the first real exp
        nc.scalar.activation(warm[0:1, 0:16], warm[0:1, 16:32], Exp, scale=1.0)

        # PE warmup: ~14 N=512 matmuls spanning the input-DMA window keep
        # the PE busy so HAM un-throttles to K=8/8 before the first real
        # matmul (which otherwise runs the whole first QK block at 1.2GHz)
        warm_ps = ps_sh.tile([128, 512], FP32, name="warm_ps", tag="sh")
        for i in range(14):
            nc.tensor.matmul(
                warm_ps, lhsT=warm[:, 0:128], rhs=warm, start=True, stop=True
            )

        # ---------- batched input loads, 3 queues, need-ordered ----------
        x_all = sb_x.tile([128, CB * N], BF16, name="x_all", tag="x")
        nc.sync.dma_start(
            x_all.rearrange("p (c n) -> p c n", c=CB)[:, :, 0:512],
            bass.AP(tensor=xT.tensor, offset=xT.offset, ap=[[N, 128], [128 * N, CB], [1, 512]]),
        )
        d_xqb1 = nc.sync.dma_start(
            x_all.rearrange("p (c n) -> p c n", c=CB)[:, :, 512:1024],
            bass.AP(tensor=xT.tensor, offset=xT.offset + 512, ap=[[N, 128], [128 * N, CB], [1, 512]]),
        )
        wqk0_t = sb_wqk0.tile([128, CB * 256], BF16, name="wqk0", tag="wqk0")
        for w in range(2):
            nc.scalar.dma_start(
                wqk0_t.rearrange("p (c w m) -> p c w m", c=CB, w=2)[:, :, w],
                bass.AP(tensor=wqkv.tensor, offset=wqkv.offset + w * D, ap=[[W3, 128], [128 * W3, CB], [1, 128]]),
            )
        # the 4.4MB of weights the first exp does NOT need: chained after
        # x qb1 so they don't steal HBM bandwidth from the critical path
        wv_all = sb_wv.tile([128, CB * D], BF16, name="wv", tag="wv")
        d_wv = nc.scalar.dma_start(
            wv_all.rearrange("p (c n) -> p c n", c=CB),
            bass.AP(tensor=wqkv.tensor, offset=wqkv.offset + 2 * D, ap=[[W3, 128], [128 * W3, CB], [1, D]]),
        )
        add_dep_helper(d_wv.ins, d_xqb1.ins, sync=True, reason="bw order wv")
        wqkr_t = sb_wqkr.tile([128, CB * 1280], BF16, name="wqkr", tag="wqkr")
        for w in range(2):
            d_wqkr = nc.gpsimd.dma_start(
                wqkr_t.rearrange("p (c w m) -> p c w m", c=CB, w=2)[:, :, w],
                bass.AP(tensor=wqkv.tensor, offset=wqkv.offset + 128 + w * D, ap=[[W3, 128], [128 * W3, CB], [1, 640]]),
            )
            add_dep_helper(d_wqkr.ins, d_xqb1.ins, sync=True, reason="bw order wqkr")
        wp_all = sb_wproj.tile([128, CB * D], BF16, name="wp", tag="wp")
        nc.gpsimd.dma_start(
            wp_all.rearrange("p (c n) -> p c n", c=CB),
            bass.AP(tensor=wproj.tensor, offset=wproj.offset, ap=[[D, 128], [128 * D, CB], [1, D]]),
        )
        nc.gpsimd.dma_start(bias_sb, bproj.rearrange("(a p) -> p a", p=128))
        s_dram = dram.tile([PAIRS, 4, 512], FP32, name="s_dram")

        def x_sl(c, lo, hi):
            return x_all[:, c * N + lo : c * N + hi]

        def wqk_slice(c, p, which):
            if p == 0:
                return wqk0_t[:, c * 256 + which * 128 : c * 256 + (which + 1) * 128]
            o = c * 1280 + which * 640 + (p - 1) * 128
            return wqkr_t[:, o : o + 128]

        def wp_slice(c, mb):
            return wp_all[:, c * D + mb * 128 : c * D + (mb + 1) * 128]

        # ---------- QK projection groups (prologue ring: sh + av slots) ----
        qk_sb = {}  # (which, pair) -> [128, N] bf16
        _ring_state = [0]

        def ring_tile(name):
            # round-robin [sh, av, av] staging slots for QK/V psum groups
            i = _ring_state[0] % 3
            _ring_state[0] += 1
            if i == 0:
                return ps_sh.tile([128, 512], FP32, name=name, tag="sh")
            return ps_av.tile([128, 512], FP32, name=name, tag="av")

        def emit_qk_group(p, which, qb, body=False, slot=None):
            if (which, p) not in qk_sb:
                qkt = sb_qk.tile([128, N], BF16, name=f"qk{which}_{p}", tag="qk")
                qk_sb[(which, p)] = qkt
            qkt = qk_sb[(which, p)]
            if slot == "den":
                ps = ps_den.tile([128, 512], FP32, name=f"qkps{which}_{p}_{qb}", tag="den")
            elif body:
                ps = ps_sh.tile([128, 512], FP32, name=f"qkps{which}_{p}_{qb}", tag="sh")
            else:
                ps = ring_tile(f"qkps{which}_{p}_{qb}")
            for c in range(CB):
                nc.tensor.matmul(
                    ps,
                    lhsT=wqk_slice(c, p, which),
                    rhs=x_sl(c, qb * 512, (qb + 1) * 512),
                    start=(c == 0),
                    stop=(c == CB - 1),
                )
            nc.vector.tensor_copy(qkt[:, qb * 512 : (qb + 1) * 512], ps)

        # ---------- V projection ----------
        v_sb = []

        def emit_v(t, sh_only=False):
            vt = sb_v.tile([128, D], BF16, name=f"v{t}", tag="v")
            for n0, nw in ((0, 512), (512, 256)):
                if sh_only:
                    vps = ps_sh.tile([128, 512], FP32, name=f"vps{t}_{n0}", tag="sh")
                else:
                    vps = ring_tile(f"vps{t}_{n0}")
                for c in range(CB):
                    nc.tensor.matmul(
                        vps[:, 0:nw],
                        lhsT=x_sl(c, t * 128, (t + 1) * 128),
                        rhs=wv_all[:, c * D + n0 : c * D + n0 + nw],
                        start=(c == 0),
                        stop=(c == CB - 1),
                    )
                nc.vector.tensor_copy(vt[:, n0 : n0 + nw], vps[:, 0:nw])
            v_sb.append(vt)

        # ---------- S^T + exp ----------
        pt_tiles = {}  # (pair, j, kt) -> [128, N] bf16

        def st_src(st):
            # one [128, N] view across the tile's two psum banks
            return bass.AP(
                tensor=st.tensor, offset=st.offset, ap=[st.ap[0], [1, N]]
            )

        def emit_st_exp(p, kt):
            # Both heads' S^T for this ktile with alternating row groups
            # (partitions 0-63 / 64-127) so consecutive matmuls overlap in
            # the PE array, then one [128,1024] exp per head on ACT.
            q_t, k_t = qk_sb[(0, p)], qk_sb[(1, p)]
            sts = []
            for j in (0, 1):
                st = ps_st.tile([128, N], FP32, name=f"st{2*p+j}_{kt}", tag="st")
                sts.append(st)
            # j-major order: both qb halves of j0 first (same row group,
            # same lhsT -> back-to-back), then j1.  Keeps exp(j0) off j1's
            # slot-wait; measured faster than qb-major co-issue attempts.
            prev_mm = None
            for j in (0, 1):
                for qb in range(QB):
                    mm = nc.tensor.matmul(
                        sts[j][:, qb * 512 : (qb + 1) * 512],
                        lhsT=k_t[j * 64 : (j + 1) * 64, kt * 128 : (kt + 1) * 128],
                        rhs=q_t[j * 64 : (j + 1) * 64, qb * 512 : (qb + 1) * 512],
                        start=True,
                        stop=True,
                    )
                    if prev_mm is not None:
                        add_dep_helper(
                            mm.ins,
                            prev_mm.ins,
                            sync=False,
                            reason="pin ST issue order",
                        )
                    prev_mm = mm
                pt = sb_pt.tile([128, N], BF16, name=f"pt{2*p+j}_{kt}", tag="pt")
                nc.scalar.activation(pt, st_src(sts[j]), Exp, scale=SCALE)
                pt_tiles[(p, j, kt)] = pt

        # ---------- denominators ----------
        den_ps = {}  # pair -> [128, 512] psum (rows 0/32/64/96 valid)

        def emit_den_kt(p, kt):
            # 4x-column-tiled ones-matmuls: all four (j, qb) denominators
            # of this ktile accumulate concurrently in one psum bank at
            # partition rows 0/32/64/96.
            if p not in den_ps:
                den_ps[p] = ps_den.tile([128, 512], FP32, name=f"den{p}", tag="den")
                if p == 0:
                    # one-time init of the never-written garbage rows so
                    # the later full-tile reciprocal reads defined data
                    nc.vector.memset(den_ps[p], 1.0)
            dps = den_ps[p]
            for j in (0, 1):
                for qb in range(QB):
                    r = 32 * (2 * qb + j)
                    nc.tensor.matmul(
                        dps[r : r + 1, :],
                        lhsT=ones_col,
                        rhs=pt_tiles[(p, j, kt)][:, qb * 512 : (qb + 1) * 512],
                        start=(kt == 0),
                        stop=(kt == KT - 1),
                        skip_group_check=True,
                        tile_position=(0, r),
                    )

        # ---------- AV (column-tiled, M=64 per head) ----------
        def emit_av_q(p, tile_, qb, kt):
            for j in (0, 1):
                h = 2 * p + j
                nc.tensor.matmul(
                    tile_[j * 64 : (j + 1) * 64, :],
                    lhsT=v_sb[kt][:, h * 64 : (h + 1) * 64],
                    rhs=pt_tiles[(p, j, kt)][:, qb * 512 : (qb + 1) * 512],
                    start=(kt == 0),
                    stop=(kt == KT - 1),
                    skip_group_check=True,
                )

        def emit_av_kt(p, av_tiles, kt, only_qb=None):
            # j-major: both qb halves of a head share the stationary V
            # tile (one LDWEIGHTS); j1's column group overlaps j0's tail
            for j in (0, 1):
                h = 2 * p + j
                for qb in range(QB):
                    if only_qb is not None and qb != only_qb:
                        continue
                    nc.tensor.matmul(
                        av_tiles[qb][j * 64 : (j + 1) * 64, :],
                        lhsT=v_sb[kt][:, h * 64 : (h + 1) * 64],
                        rhs=pt_tiles[(p, j, kt)][:, qb * 512 : (qb + 1) * 512],
                        start=(kt == 0),
                        stop=(kt == KT - 1),
                        skip_group_check=True,
                    )

        # ---------- normalization ----------
        def emit_recip(p):
            # reciprocal straight from the den psum bank (rows 0/32/64/96
            # valid; garbage rows were memset to 1.0 once in the prologue).
            dps = den_ps[p]
            rsb = sb_den.tile([128, 512], FP32, name=f"rsb{p}", tag="rsb")
            nc.vector.reciprocal_approx_fast(rsb, dps)
            return rsb

        def emit_norm_bcast(p, rsb):
            # DRAM round-trip broadcast: rsb rows -> s_dram[p] -> rbc tile
            # [128, 1024] (rows 0-63 = 1/d(j0), rows 64-127 = 1/d(j1))
            sd = s_dram[p]
            for i in range(4):
                nc.sync.dma_start(sd[i], rsb[32 * i : 32 * i + 1, :])
            rbc = sb_rbc.tile([128, N], FP32, name=f"rbc{p}", tag="rbc")
            for j in (0, 1):
                row = sd[j]
                bcast = bass.AP(
                    tensor=row.tensor,
                    offset=row.offset,
                    ap=[[0, 64], [2 * 512, 2], [1, 512]],
                )
                nc.gpsimd.dma_start(
                    rbc[j * 64 : (j + 1) * 64, :].rearrange(
                        "p (a b) -> p a b", a=2
                    ),
                    bcast,
                )
            return rbc

        def emit_norm_mul(p, av_tiles, rbc, at):
            for qb in range(QB):
                nc.vector.tensor_mul(
                    at[:, qb * 512 : (qb + 1) * 512],
                    av_tiles[qb],
                    rbc[:, qb * 512 : (qb + 1) * 512],
                )

        # ---------- output projection ----------
        at_sb = []

        def emit_proj_k(ps_pair, mb, c_lo, c_hi):
            for qb in range(QB):
                for c in range(c_lo, c_hi):
                    nc.tensor.matmul(
                        ps_pair[qb],
                        lhsT=wp_slice(c, mb),
                        rhs=at_sb[c][:, qb * 512 : (qb + 1) * 512],
                        start=(c == 0),
                        stop=(c == CB - 1),
                    )

        def emit_proj_out(ps_pair, mb, on_act=True):
            # bias-add eviction + bf16 store; qb1 evicts on the (idle) ACT
            ot = sb_out.tile([128, N], BF16, name=f"out{mb}", tag="out")
            for qb in range(QB):
                dst = ot[:, qb * 512 : (qb + 1) * 512]
                if on_act and qb == 1:
                    nc.scalar.add(dst, ps_pair[qb], bias_sb[:, mb : mb + 1])
                else:
                    nc.vector.tensor_scalar_add(
                        dst, ps_pair[qb], bias_sb[:, mb : mb + 1]
                    )
                nc.sync.dma_start(
                    out[mb * 128 : (mb + 1) * 128, qb * 512 : (qb + 1) * 512],
                    dst,
                )

        # =======================================================
        # prologue: QK(0) 4-way, then per-kt [ST(0)+exp(0), den(0) lag-2,
        # V(0..5), QK(1)].  den(0) kt6/7, V(6/7) wrap into body 0.
        # =======================================================
        emit_qk_group(0, 0, 0)
        emit_qk_group(0, 1, 0)
        emit_qk_group(0, 0, 1)
        emit_qk_group(0, 1, 1, slot="den")
        for kt in range(KT):
            emit_st_exp(0, kt)
            if kt >= 2:
                emit_den_kt(0, kt - 2)
            if kt < 6:
                emit_v(kt)
            if kt >= 4:
                qb_, which_ = divmod(kt - 4, 2)
                emit_qk_group(1, which_, qb_)

        # =======================================================
        # pipelined pairs.  body(p) steady state per kt:
        #   [ST(p+1,kt), den(p+1,kt-2), AV(p,kt-2), QK(p+2)]
        # kt6/7 leftovers of den(p+1)/AV(p) WRAP into body(p+1) steps 0/1
        # so pair boundaries never burst (keeps the exp stream gapless).
        # norm(p-1) runs at body(p) step 1 once its wrapped AV lands.
        # AV(5,qb0) accumulates in the sh slot during body 4 (QK is done),
        # so body 5 only owes AV(5,qb1) + norm + proj.
        # =======================================================
        prev_av = None
        prev_rbc = None
        av5_q0 = None
        for p in range(PAIRS):
            last = p == PAIRS - 1
            at = sb_at.tile([128, N], BF16, name=f"attn{p}", tag="attn")
            at_sb.append(at)
            if last:
                av_tiles = [
                    av5_q0,
                    ps_av.tile([128, 512], FP32, name="av5_1", tag="av"),
                ]
                mb01_ps = []
            else:
                av_tiles = [
                    ps_av.tile([128, 512], FP32, name=f"av{p}_{qb}", tag="av")
                    for qb in range(QB)
                ]

            for kt in range(KT):
                if not last:
                    emit_st_exp(p + 1, kt)
                if p == 0 and kt < 2:
                    emit_v(KT - 2 + kt, sh_only=True)
                if kt < 2:
                    # wrapped leftovers from the previous body
                    emit_den_kt(p, KT - 2 + kt)
                    if p >= 1:
                        emit_av_kt(p - 1, prev_av, KT - 2 + kt)
                        if p == 5:
                            emit_av_q(5, av5_q0, 0, KT - 2 + kt)
                    if kt == 1:
                        rsb = emit_recip(p)
                        if p >= 1:
                            emit_norm_mul(p - 1, prev_av, prev_rbc, at_sb[p - 1])
                        rbc = emit_norm_bcast(p, rsb)
                else:
                    if not last:
                        emit_den_kt(p + 1, kt - 2)
                    # pair 5's qb0 was already accumulated during body 4
                    emit_av_kt(p, av_tiles, kt - 2, only_qb=1 if last else None)
                    if p == 4:
                        if kt == 2:
                            av5_q0 = ps_sh.tile(
                                [128, 512], FP32, name="av5_0", tag="sh"
                            )
                        emit_av_q(5, av5_q0, 0, kt - 2)
                if p + 2 < PAIRS and 2 <= kt <= 5:
                    qb_, which_ = divmod(kt - 2, 2)
                    emit_qk_group(p + 2, which_, qb_, body=True)
                if last and kt == 1:
                    t0 = ps_st.tile([128, N], FP32, name="projps0", tag="st")
                    mb01_ps.append([t0[:, 0:512], t0[:, 512:1024]])
                    emit_proj_k(mb01_ps[0], 0, 0, CB - 1)
                if last and kt == 2:
                    t1 = ps_st.tile([128, N], FP32, name="projps1", tag="st")
                    mb01_ps.append([t1[:, 0:512], t1[:, 512:1024]])
                    emit_proj_k(mb01_ps[1], 1, 0, CB - 1)
                if last and kt == 3:
                    mb2q0 = ps_den.tile(
                        [128, 512], FP32, name="projps2a", tag="den"
                    )
                    for c in range(CB - 1):
                        nc.tensor.matmul(
                            mb2q0,
                            lhsT=wp_slice(c, 2),
                            rhs=at_sb[c][:, 0:512],
                            start=(c == 0),
                            stop=False,
                        )
            if last:
                # finish AV(5,qb0) kt6/7 (sh) happened in wrap above; qb1:
                emit_av_kt(p, av_tiles, KT - 2, only_qb=1)
                emit_av_kt(p, av_tiles, KT - 1, only_qb=1)
                emit_norm_mul(p, av_tiles, rbc, at)
            else:
                prev_av = av_tiles
                prev_rbc = rbc

        # =======================================================
        # projection tail
        # =======================================================
        emit_proj_k(mb01_ps[0], 0, CB - 1, CB)
        emit_proj_out(mb01_ps[0], 0)
        emit_proj_k(mb01_ps[1], 1, CB - 1, CB)
        emit_proj_out(mb01_ps[1], 1)
        nc.tensor.matmul(
            mb2q0,
            lhsT=wp_slice(CB - 1, 2),
            rhs=at_sb[CB - 1][:, 0:512],
            start=False,
            stop=True,
        )
        mb2q1 = ps_av.tile([128, 512], FP32, name="projps2b", tag="av")
        for c in range(CB):
            nc.tensor.matmul(
                mb2q1,
                lhsT=wp_slice(c, 2),
                rhs=at_sb[c][:, 512:1024],
                start=(c == 0),
                stop=(c == CB - 1),
            )
        emit_proj_out([mb2q0, mb2q1], 2)
        t3 = ps_st.tile([128, N], FP32, name="projps3", tag="st")
        mb3_ps = [t3[:, 0:512], t3[:, 512:1024]]
        emit_proj_k(mb3_ps, 3, 0, CB)
        emit_proj_out(mb3_ps, 3)
        t4 = ps_st.tile([128, N], FP32, name="projps4", tag="st")
        mb4_ps = [t4[:, 0:512], t4[:, 512:1024]]
        emit_proj_k(mb4_ps, 4, 0, CB)
        emit_proj_out(mb4_ps, 4)
        mb5a = ps_sh.tile([128, 512], FP32, name="projps5a", tag="sh")
        mb5b = ps_den.tile([128, 512], FP32, name="projps5b", tag="den")
        mb5_ps = [mb5a, mb5b]
        emit_proj_k(mb5_ps, 5, 0, CB)
        emit_proj_out(mb5_ps, 5)


def build_nc():
    nc = bacc.Bacc(
        "TRN2", target_bir_lowering=False, debug=False, num_devices=NCORES
    )
    ins = {
        "xT": nc.dram_tensor("xT", [D, N], BF16, kind="ExternalInput").ap(),
        "w_qkv": nc.dram_tensor("w_qkv", [D, 3 * D], BF16, kind="ExternalInput").ap(),
        "w_proj": nc.dram_tensor("w_proj", [D, D], BF16, kind="ExternalInput").ap(),
        "b_proj": nc.dram_tensor("b_proj", [D], FP32, kind="ExternalInput").ap(),
    }
    outs = {"out": nc.dram_tensor("out", [D, N], BF16, kind="ExternalOutput").ap()}
    with tile.TileContext(nc) as tc:
        build_attention(tc, outs, ins)
    nc.compile()
    return nc


def make_in_maps(x, w_qkv, w_proj, b_proj):
    xT = np.ascontiguousarray(
        np.transpose(np.asarray(x, np.float32), (0, 2, 1))
    ).astype(ml_dtypes.bfloat16)
    wq = np.asarray(w_qkv, np.float32).astype(ml_dtypes.bfloat16)
    wp = np.asarray(w_proj, np.float32).astype(ml_dtypes.bfloat16)
    bp = np.ascontiguousarray(np.asarray(b_proj, np.float32))
    return [
        {"xT": np.ascontiguousarray(xT[b]), "w_qkv": wq, "w_proj": wp, "b_proj": bp}
        for b in range(B)
    ]


_BUILT = None


def _get_built():
    global _BUILT
    if _BUILT is None:
        _BUILT = build_nc()
    return _BUILT


def kernel(x, w_qkv, w_proj, b_proj):
    from concourse.bass_utils import run_bass_kernel_spmd

    nc = _get_built()
    in_maps = make_in_maps(x, w_qkv, w_proj, b_proj)
    res = run_bass_kernel_spmd(nc, in_maps, core_ids=list(range(NCORES)))
    return np.stack(
        [
            np.asarray(res.results[b]["out"], np.float32).T
            for b in range(B)
        ]
    )


# revision 14
# speedup vs baseline: 1.0336x; 1.0136x over previous
"""Multi-head attention (B=8, N=1024, D=768, H=12) on 8 TRN2 NeuronCores.

Sharding: data-parallel over batch B - one batch element per core, weights
replicated, no collectives.

Design (v5 = v2 + faster prologue): the kernel is a two-engine race
between the PE (matmuls, ~115us) and ACT (96 exps of [128,1024], ~110us).
Everything else hides.  v5 changes vs v2: (1) input DMA batched into 9
large transfers across 3 queues, with the 4.4MB of weights the first exp
does NOT need dep-chained behind x qb1 so they don't steal HBM bandwidth
from the exp-critical 1.9MB; (2) 14 PE warmup matmuls spanning the DMA
window so HAM holds K=8/8 into the first QK matmuls.

Per-core layout (feature-major, no on-chip transposes):
  x^T [768, 1024] (host-transposed, bf16)
  Q/K feature-major [c, n]: lhsT = w_qkv block, rhs = x^T          -> QK_fm
  V token-major  [n, c]:    lhsT = x^T block,  rhs = w_qkv V cols  -> V_tm
     (plain [128, 768] tiles - no ones column)
  S^T[k, q] per (pair, j, ktile): lhsT = K_fm [64,128], rhs = Q_fm [64,512]
     row-tiled j0/j1 (partitions 0-63 / 64-127) for PE-array overlap
  P^T = exp(SCALE * S^T) on ACT, bf16, one [128,1024] instr per (j, kt)
  AV: column-tiled pure-V matmuls, M=64 per head: j0 -> psum rows 0-63,
     j1 -> rows 64-127 (tile_position (0,0)/(0,64)), accumulated over kt
     into av_ps[qb] [128,512].  Softmax denominators via a separate
     4x-column-tiled ones-matmul pass: lhsT = ones[128,1], out rows
     0/32/64/96 of one psum bank = (j0,qb0),(j1,qb0),(j0,qb1),(j1,qb1).
     Shared-bank accumulation groups are safe: start=True clears
     has_written only for the matmul's own output region (measured), so
     each region's first MM carries start=True.
  norm: recip of the 4 denominator rows (strided-partition DVE), DRAM
     round-trip broadcast to rbc [128,1024] (pairs 0-4), then one
     tensor_mul(av_psum, rbc_sbuf) -> at per qb.  Pair 5 uses the same
     DRAM path issued at body-5 step 1.
  proj: lhsT = w_proj block, rhs = at -> out_fm [768,1024] bf16 + bias
     (bias folded into the DVE/ACT eviction).  mb0/1 prefill c0-c4 in ST
     slots during pair 5, mb2qb0 in the den slot; mb3-5 run in the tail.
Host gathers out_fm (bf16), upcasts, transposes back to [B, 1024, 768].

PSUM budget (8 banks): st 2x[128,1024]=4, av 2x[128,512]=2, den 1, sh 1.
In the prologue the av ring doubles as QK/V eviction staging; in the
pair loop QK(p+2) uses the single sh slot.  PE warmup matmuls run during
the initial DMA window so HAM reaches K=8/8 before the first real MM.
"""

import numpy as np
import ml_dtypes

import concourse.bass as bass
import concourse.tile as tile
from concourse import bacc, mybir
from concourse.tile import add_dep_helper

FP32 = mybir.dt.float32
BF16 = mybir.dt.bfloat16

B, N, D = 8, 1024, 768
H, HD = 12, 64
SCALE = float(HD) ** -0.5  # 0.125
CB = D // 128  # 6 contraction blocks of 128
PAIRS = H // 2  # 6 head pairs
KT = N // 128  # 8 key-token tiles
QB = N // 512  # 2 q blocks of 512
NCORES = 8
W3 = 3 * D  # 2304


def build_attention(tc, outs, ins):
    from contextlib import ExitStack

    nc = tc.nc
    xT = ins["xT"]  # [768, 1024] bf16 dram
    wqkv = ins["w_qkv"]  # [768, 2304] bf16 dram
    wproj = ins["w_proj"]  # [768, 768] bf16 dram
    bproj = ins["b_proj"]  # [768] fp32 dram
    out = outs["out"]  # [768, 1024] bf16 dram

    Exp = mybir.ActivationFunctionType.Exp

    with ExitStack() as ctx:
        ec = ctx.enter_context
        sb_x = ec(tc.tile_pool(name="sb_x", bufs=1))
        sb_wqk0 = ec(tc.tile_pool(name="sb_wqk0", bufs=1))
        sb_wqkr = ec(tc.tile_pool(name="sb_wqkr", bufs=1))
        sb_wv = ec(tc.tile_pool(name="sb_wv", bufs=1))
        sb_wproj = ec(tc.tile_pool(name="sb_wproj", bufs=1))
        sb_misc = ec(tc.tile_pool(name="sb_misc", bufs=1))
        sb_qk = ec(tc.tile_pool(name="sb_qk", bufs=6))
        sb_v = ec(tc.tile_pool(name="sb_v", bufs=KT))
        sb_pt = ec(tc.tile_pool(name="sb_pt", bufs=40))
        sb_den = ec(tc.tile_pool(name="sb_den", bufs=2))
        sb_rbc = ec(tc.tile_pool(name="sb_rbc", bufs=2))
        sb_at = ec(tc.tile_pool(name="sb_at", bufs=PAIRS))
        sb_stage = ec(tc.tile_pool(name="sb_stage", bufs=2))
        sb_out = ec(tc.tile_pool(name="sb_out", bufs=3))
        ps_st = ec(tc.tile_pool(name="ps_st", bufs=2, space="PSUM"))
        ps_av = ec(tc.tile_pool(name="ps_av", bufs=2, space="PSUM"))
        ps_den = ec(tc.tile_pool(name="ps_den", bufs=1, space="PSUM"))
        ps_sh = ec(tc.tile_pool(name="ps_sh", bufs=1, space="PSUM"))
        dram = ec(tc.tile_pool(name="dram", bufs=1, space="DRAM"))

        # ---------- constants / warmup ----------
        warm = sb_misc.tile([128, 512], BF16, name="warm", tag="warm")
        nc.vector.memset(warm, 0.0)
        ones_col = sb_misc.tile([128, 1], BF16, name="ones_col", tag="onesc")
        nc.vector.memset(ones_col, 1.0)
        ones_row = sb_misc.tile([128, 64], BF16, name="ones_row", tag="onesr")
        nc.vector.memset(ones_row, 1.0)
        bias_sb = sb_misc.tile([128, CB], FP32, name="bias")

        # trigger the ACT exp table load (~2.7us) during the initial DMA
        # window instead of serializing it before the first real exp
        nc.scalar.activation(warm[0:1, 0:16], warm[0:1, 16:32], Exp, scale=1.0)

        # PE warmup: ~14 N=512 matmuls spanning the input-DMA window keep
        # the PE busy so HAM un-throttles to K=8/8 before the first real
        # matmul (which otherwise runs the whole first QK block at 1.2GHz)
        warm_ps = ps_sh.tile([128, 512], FP32, name="warm_ps", tag="sh")
        for i in range(14):
            nc.tensor.matmul(
                warm_ps, lhsT=warm[:, 0:128], rhs=warm, start=True, stop=True
            )

        # ---------- batched input loads, 3 queues, need-ordered ----------
        x_all = sb_x.tile([128, CB * N], BF16, name="x_all", tag="x")
        nc.sync.dma_start(
            x_all.rearrange("p (c n) -> p c n", c=CB)[:, :, 0:512],
            bass.AP(tensor=xT.tensor, offset=xT.offset, ap=[[N, 128], [128 * N, CB], [1, 512]]),
        )
        d_xqb1 = nc.sync.dma_start(
            x_all.rearrange("p (c n) -> p c n", c=CB)[:, :, 512:1024],
            bass.AP(tensor=xT.tensor, offset=xT.offset + 512, ap=[[N, 128], [128 * N, CB], [1, 512]]),
        )
        wqk0_t = sb_wqk0.tile([128, CB * 256], BF16, name="wqk0", tag="wqk0")
        for w in range(2):
            nc.scalar.dma_start(
                wqk0_t.rearrange("p (c w m) -> p c w m", c=CB, w=2)[:, :, w],
                bass.AP(tensor=wqkv.tensor, offset=wqkv.offset + w * D, ap=[[W3, 128], [128 * W3, CB], [1, 128]]),
            )
        # the 4.4MB of weights the first exp does NOT need: chained after
        # x qb1 so they don't steal HBM bandwidth from the critical path
        wv_all = sb_wv.tile([128, CB * D], BF16, name="wv", tag="wv")
        d_wv = nc.scalar.dma_start(
            wv_all.rearrange("p (c n) -> p c n", c=CB),
            bass.AP(tensor=wqkv.tensor, offset=wqkv.offset + 2 * D, ap=[[W3, 128], [128 * W3, CB], [1, D]]),
        )
        add_dep_helper(d_wv.ins, d_xqb1.ins, sync=True, reason="bw order wv")
        wqkr_t = sb_wqkr.tile([128, CB * 1280], BF16, name="wqkr", tag="wqkr")
        for w in range(2):
            d_wqkr = nc.gpsimd.dma_start(
                wqkr_t.rearrange("p (c w m) -> p c w m", c=CB, w=2)[:, :, w],
                bass.AP(tensor=wqkv.tensor, offset=wqkv.offset + 128 + w * D, ap=[[W3, 128], [128 * W3, CB], [1, 640]]),
            )
            add_dep_helper(d_wqkr.ins, d_xqb1.ins, sync=True, reason="bw order wqkr")
        wp_all = sb_wproj.tile([128, CB * D], BF16, name="wp", tag="wp")
        nc.gpsimd.dma_start(
            wp_all.rearrange("p (c n) -> p c n", c=CB),
            bass.AP(tensor=wproj.tensor, offset=wproj.offset, ap=[[D, 128], [128 * D, CB], [1, D]]),
        )
        nc.gpsimd.dma_start(bias_sb, bproj.rearrange("(a p) -> p a", p=128))
        s_dram = dram.tile([PAIRS, 4, 512], FP32, name="s_dram")

        def x_sl(c, lo, hi):
            return x_all[:, c * N + lo : c * N + hi]

        def wqk_slice(c, p, which):
            if p == 0:
                return wqk0_t[:, c * 256 + which * 128 : c * 256 + (which + 1) * 128]
            o = c * 1280 + which * 640 + (p - 1) * 128
            return wqkr_t[:, o : o + 128]

        def wp_slice(c, mb):
            return wp_all[:, c * D + mb * 128 : c * D + (mb + 1) * 128]

        # ---------- QK projection groups (prologue ring: sh + av slots) ----
        qk_sb = {}  # (which, pair) -> [128, N] bf16
        _ring_state = [0]

        def ring_tile(name):
            # round-robin [sh, av, av] staging slots for QK/V psum groups
            i = _ring_state[0] % 3
            _ring_state[0] += 1
            if i == 0:
                return ps_sh.tile([128, 512], FP32, name=name, tag="sh")
            return ps_av.tile([128, 512], FP32, name=name, tag="av")

        def emit_qk_group(p, which, qb, body=False, slot=None):
            if (which, p) not in qk_sb:
                qkt = sb_qk.tile([128, N], BF16, name=f"qk{which}_{p}", tag="qk")
                qk_sb[(which, p)] = qkt
            qkt = qk_sb[(which, p)]
            if slot == "den":
                ps = ps_den.tile([128, 512], FP32, name=f"qkps{which}_{p}_{qb}", tag="den")
            elif body:
                ps = ps_sh.tile([128, 512], FP32, name=f"qkps{which}_{p}_{qb}", tag="sh")
            else:
                ps = ring_tile(f"qkps{which}_{p}_{qb}")
            for c in range(CB):
                nc.tensor.matmul(
                    ps,
                    lhsT=wqk_slice(c, p, which),
                    rhs=x_sl(c, qb * 512, (qb + 1) * 512),
                    start=(c == 0),
                    stop=(c == CB - 1),
                )
            nc.vector.tensor_copy(qkt[:, qb * 512 : (qb + 1) * 512], ps)

        # ---------- V projection ----------
        v_sb = []

        def emit_v(t, sh_only=False):
            vt = sb_v.tile([128, D], BF16, name=f"v{t}", tag="v")
            for n0, nw in ((0, 512), (512, 256)):
                if sh_only:
                    vps = ps_sh.tile([128, 512], FP32, name=f"vps{t}_{n0}", tag="sh")
                else:
                    vps = ring_tile(f"vps{t}_{n0}")
                for c in range(CB):
                    nc.tensor.matmul(
                        vps[:, 0:nw],
                        lhsT=x_sl(c, t * 128, (t + 1) * 128),
                        rhs=wv_all[:, c * D + n0 : c * D + n0 + nw],
                        start=(c == 0),
                        stop=(c == CB - 1),
                    )
                nc.vector.tensor_copy(vt[:, n0 : n0 + nw], vps[:, 0:nw])
            v_sb.append(vt)

        # ---------- S^T + exp ----------
        pt_tiles = {}  # (pair, j, kt) -> [128, N] bf16

        def st_src(st):
            # one [128, N] view across the tile's two psum banks
            return bass.AP(
                tensor=st.tensor, offset=st.offset, ap=[st.ap[0], [1, N]]
            )

        def emit_st_exp(p, kt):
            # Both heads' S^T for this ktile with alternating row groups
            # (partitions 0-63 / 64-127) so consecutive matmuls overlap in
            # the PE array, then one [128,1024] exp per head on ACT.
            q_t, k_t = qk_sb[(0, p)], qk_sb[(1, p)]
            sts = []
            for j in (0, 1):
                st = ps_st.tile([128, N], FP32, name=f"st{2*p+j}_{kt}", tag="st")
                sts.append(st)
            # j-major order: both qb halves of j0 first (same row group,
            # same lhsT -> back-to-back), then j1.  Keeps exp(j0) off j1's
            # slot-wait; measured faster than qb-major co-issue attempts.
            prev_mm = None
            for j in (0, 1):
                for qb in range(QB):
                    mm = nc.tensor.matmul(
                        sts[j][:, qb * 512 : (qb + 1) * 512],
                        lhsT=k_t[j * 64 : (j + 1) * 64, kt * 128 : (kt + 1) * 128],
                        rhs=q_t[j * 64 : (j + 1) * 64, qb * 512 : (qb + 1) * 512],
                        start=True,
                        stop=True,
                    )
                    if prev_mm is not None:
                        add_dep_helper(
                            mm.ins,
                            prev_mm.ins,
                            sync=False,
                            reason="pin ST issue order",
                        )
                    prev_mm = mm
                pt = sb_pt.tile([128, N], BF16, name=f"pt{2*p+j}_{kt}", tag="pt")
                nc.scalar.activation(pt, st_src(sts[j]), Exp, scale=SCALE)
                pt_tiles[(p, j, kt)] = pt

        # ---------- denominators ----------
        den_ps = {}  # pair -> [128, 512] psum (rows 0/32/64/96 valid)

        def emit_den_kt(p, kt):
            # 4x-column-tiled ones-matmuls: all four (j, qb) denominators
            # of this ktile accumulate concurrently in one psum bank at
            # partition rows 0/32/64/96.
            if p not in den_ps:
                den_ps[p] = ps_den.tile([128, 512], FP32, name=f"den{p}", tag="den")
                if p == 0:
                    # one-time init of the never-written garbage rows so
                    # the later full-tile reciprocal reads defined data
                    nc.vector.memset(den_ps[p], 1.0)
            dps = den_ps[p]
            for j in (0, 1):
                for qb in range(QB):
                    r = 32 * (2 * qb + j)
                    nc.tensor.matmul(
                        dps[r : r + 1, :],
                        lhsT=ones_col,
                        rhs=pt_tiles[(p, j, kt)][:, qb * 512 : (qb + 1) * 512],
                        start=(kt == 0),
                        stop=(kt == KT - 1),
                        skip_group_check=True,
                        tile_position=(0, r),
                    )

        # ---------- AV (4x column-striped, M=32 per strip) ----------
        def _av_mms(p, dst, qb, kt):
            # all four (j, dh) M=32 strips stream concurrently through
            # separate 32-wide column groups (same mechanism as den)
            for j in (0, 1):
                h = 2 * p + j
                for dh in (0, 1):
                    col = j * 64 + dh * 32
                    nc.tensor.matmul(
                        dst[col : col + 32, :],
                        lhsT=v_sb[kt][:, h * 64 + dh * 32 : h * 64 + dh * 32 + 32],
                        rhs=pt_tiles[(p, j, kt)][:, qb * 512 : (qb + 1) * 512],
                        start=(kt == 0),
                        stop=(kt == KT - 1),
                        skip_group_check=True,
                        tile_position=(0, col),
                    )

        def emit_av_q(p, tile_, qb, kt):
            _av_mms(p, tile_, qb, kt)

        def emit_av_kt(p, av_tiles, kt, only_qb=None):
            # qb-major so each qb's four strips issue back-to-back and
            # run as one concurrent 4-way group
            for qb in range(QB):
                if only_qb is not None and qb != only_qb:
                    continue
                _av_mms(p, av_tiles[qb], qb, kt)

        # ---------- normalization ----------
        def emit_recip(p):
            # reciprocal straight from the den psum bank (rows 0/32/64/96
            # valid; garbage rows were memset to 1.0 once in the prologue).
            dps = den_ps[p]
            rsb = sb_den.tile([128, 512], FP32, name=f"rsb{p}", tag="rsb")
            nc.vector.reciprocal_approx_fast(rsb, dps)
            return rsb

        def emit_norm_bcast(p, rsb):
            # DRAM round-trip broadcast: rsb rows -> s_dram[p] -> rbc tile
            # [128, 1024] (rows 0-63 = 1/d(j0), rows 64-127 = 1/d(j1))
            sd = s_dram[p]
            for i in range(4):
                nc.sync.dma_start(sd[i], rsb[32 * i : 32 * i + 1, :])
            rbc = sb_rbc.tile([128, N], FP32, name=f"rbc{p}", tag="rbc")
            for j in (0, 1):
                row = sd[j]
                bcast = bass.AP(
                    tensor=row.tensor,
                    offset=row.offset,
                    ap=[[0, 64], [2 * 512, 2], [1, 512]],
                )
                nc.gpsimd.dma_start(
                    rbc[j * 64 : (j + 1) * 64, :].rearrange(
                        "p (a b) -> p a b", a=2
                    ),
                    bcast,
                )
            return rbc

        def emit_norm_mul(p, av_tiles, rbc, at):
            for qb in range(QB):
                nc.vector.tensor_mul(
                    at[:, qb * 512 : (qb + 1) * 512],
                    av_tiles[qb],
                    rbc[:, qb * 512 : (qb + 1) * 512],
                )

        # ---------- output projection ----------
        at_sb = []

        def emit_proj_k(ps_pair, mb, c_lo, c_hi):
            for qb in range(QB):
                for c in range(c_lo, c_hi):
                    nc.tensor.matmul(
                        ps_pair[qb],
                        lhsT=wp_slice(c, mb),
                        rhs=at_sb[c][:, qb * 512 : (qb + 1) * 512],
                        start=(c == 0),
                        stop=(c == CB - 1),
                    )

        def emit_proj_out(ps_pair, mb, on_act=True):
            # bias-add eviction + bf16 store; qb1 evicts on the (idle) ACT
            ot = sb_out.tile([128, N], BF16, name=f"out{mb}", tag="out")
            for qb in range(QB):
                dst = ot[:, qb * 512 : (qb + 1) * 512]
                if on_act and qb == 1:
                    nc.scalar.add(dst, ps_pair[qb], bias_sb[:, mb : mb + 1])
                else:
                    nc.vector.tensor_scalar_add(
                        dst, ps_pair[qb], bias_sb[:, mb : mb + 1]
                    )
                nc.sync.dma_start(
                    out[mb * 128 : (mb + 1) * 128, qb * 512 : (qb + 1) * 512],
                    dst,
                )

        # =======================================================
        # prologue: QK(0) 4-way, then per-kt [ST(0)+exp(0), den(0) lag-2,
        # V(0..5), QK(1)].  den(0) kt6/7, V(6/7) wrap into body 0.
        # =======================================================
        emit_qk_group(0, 0, 0)
        emit_qk_group(0, 1, 0)
        emit_qk_group(0, 0, 1)
        emit_qk_group(0, 1, 1, slot="den")
        for kt in range(KT):
            emit_st_exp(0, kt)
            if kt >= 2:
                emit_den_kt(0, kt - 2)
            if kt < 6:
                emit_v(kt)
            if kt >= 4:
                qb_, which_ = divmod(kt - 4, 2)
                emit_qk_group(1, which_, qb_)

        # =======================================================
        # pipelined pairs.  body(p) steady state per kt:
        #   [ST(p+1,kt), den(p+1,kt-2), AV(p,kt-2), QK(p+2)]
        # kt6/7 leftovers of den(p+1)/AV(p) WRAP into body(p+1) steps 0/1
        # so pair boundaries never burst (keeps the exp stream gapless).
        # norm(p-1) runs at body(p) step 1 once its wrapped AV lands.
        # AV(5,qb0) accumulates in the sh slot during body 4 (QK is done),
        # so body 5 only owes AV(5,qb1) + norm + proj.
        # =======================================================
        prev_av = None
        prev_rbc = None
        av5_q0 = None
        for p in range(PAIRS):
            last = p == PAIRS - 1
            at = sb_at.tile([128, N], BF16, name=f"attn{p}", tag="attn")
            at_sb.append(at)
            if last:
                av_tiles = [
                    av5_q0,
                    ps_av.tile([128, 512], FP32, name="av5_1", tag="av"),
                ]
                mb01_ps = []
            else:
                av_tiles = [
                    ps_av.tile([128, 512], FP32, name=f"av{p}_{qb}", tag="av")
                    for qb in range(QB)
                ]

            for kt in range(KT):
                if not last:
                    emit_st_exp(p + 1, kt)
                if p == 0 and kt < 2:
                    emit_v(KT - 2 + kt, sh_only=True)
                if kt < 2:
                    # wrapped leftovers from the previous body
                    emit_den_kt(p, KT - 2 + kt)
                    if p >= 1:
                        emit_av_kt(p - 1, prev_av, KT - 2 + kt)
                        if p == 5:
                            emit_av_q(5, av5_q0, 0, KT - 2 + kt)
                    if kt == 1:
                        rsb = emit_recip(p)
                        if p >= 1:
                            emit_norm_mul(p - 1, prev_av, prev_rbc, at_sb[p - 1])
                        rbc = emit_norm_bcast(p, rsb)
                else:
                    if not last:
                        emit_den_kt(p + 1, kt - 2)
                    # pair 5's qb0 was already accumulated during body 4
                    emit_av_kt(p, av_tiles, kt - 2, only_qb=1 if last else None)
                    if p == 4:
                        if kt == 2:
                            av5_q0 = ps_sh.tile(
                                [128, 512], FP32, name="av5_0", tag="sh"
                            )
                        emit_av_q(5, av5_q0, 0, kt - 2)
                if p + 2 < PAIRS and 2 <= kt <= 5:
                    qb_, which_ = divmod(kt - 2, 2)
                    emit_qk_group(p + 2, which_, qb_, body=True)
                if last and kt == 1:
                    t0 = ps_st.tile([128, N], FP32, name="projps0", tag="st")
                    mb01_ps.append([t0[:, 0:512], t0[:, 512:1024]])
                    emit_proj_k(mb01_ps[0], 0, 0, CB - 1)
                if last and kt == 2:
                    t1 = ps_st.tile([128, N], FP32, name="projps1", tag="st")
                    mb01_ps.append([t1[:, 0:512], t1[:, 512:1024]])
                    emit_proj_k(mb01_ps[1], 1, 0, CB - 1)
                if last and kt == 3:
                    mb2q0 = ps_den.tile(
                        [128, 512], FP32, name="projps2a", tag="den"
                    )
                    for c in range(CB - 1):
                        nc.tensor.matmul(
                            mb2q0,
                            lhsT=wp_slice(c, 2),
                            rhs=at_sb[c][:, 0:512],
                            start=(c == 0),
                            stop=False,
                        )
            if last:
                # finish AV(5,qb0) kt6/7 (sh) happened in wrap above; qb1:
                emit_av_kt(p, av_tiles, KT - 2, only_qb=1)
                emit_av_kt(p, av_tiles, KT - 1, only_qb=1)
                emit_norm_mul(p, av_tiles, rbc, at)
            else:
                prev_av = av_tiles
                prev_rbc = rbc

        # =======================================================
        # projection tail
        # =======================================================
        emit_proj_k(mb01_ps[0], 0, CB - 1, CB)
        emit_proj_out(mb01_ps[0], 0)
        emit_proj_k(mb01_ps[1], 1, CB - 1, CB)
        emit_proj_out(mb01_ps[1], 1)
        nc.tensor.matmul(
            mb2q0,
            lhsT=wp_slice(CB - 1, 2),
            rhs=at_sb[CB - 1][:, 0:512],
            start=False,
            stop=True,
        )
        mb2q1 = ps_av.tile([128, 512], FP32, name="projps2b", tag="av")
        for c in range(CB):
            nc.tensor.matmul(
                mb2q1,
                lhsT=wp_slice(c, 2),
                rhs=at_sb[c][:, 512:1024],
                start=(c == 0),
                stop=(c == CB - 1),
            )
        emit_proj_out([mb2q0, mb2q1], 2)
        t3 = ps_st.tile([128, N], FP32, name="projps3", tag="st")
        mb3_ps = [t3[:, 0:512], t3[:, 512:1024]]
        emit_proj_k(mb3_ps, 3, 0, CB)
        emit_proj_out(mb3_ps, 3)
        t4 = ps_st.tile([128, N], FP32, name="projps4", tag="st")
        mb4_ps = [t4[:, 0:512], t4[:, 512:1024]]
        emit_proj_k(mb4_ps, 4, 0, CB)
        emit_proj_out(mb4_ps, 4)
        mb5a = ps_sh.tile([128, 512], FP32, name="projps5a", tag="sh")
        mb5b = ps_den.tile([128, 512], FP32, name="projps5b", tag="den")
        mb5_ps = [mb5a, mb5b]
        emit_proj_k(mb5_ps, 5, 0, CB)
        emit_proj_out(mb5_ps, 5)


def build_nc():
    nc = bacc.Bacc(
        "TRN2", target_bir_lowering=False, debug=False, num_devices=NCORES
    )
    ins = {
        "xT": nc.dram_tensor("xT", [D, N], BF16, kind="ExternalInput").ap(),
        "w_qkv": nc.dram_tensor("w_qkv", [D, 3 * D], BF16, kind="ExternalInput").ap(),
        "w_proj": nc.dram_tensor("w_proj", [D, D], BF16, kind="ExternalInput").ap(),
        "b_proj": nc.dram_tensor("b_proj", [D], FP32, kind="ExternalInput").ap(),
    }
    outs = {"out": nc.dram_tensor("out", [D, N], BF16, kind="ExternalOutput").ap()}
    with tile.TileContext(nc) as tc:
        build_attention(tc, outs, ins)
    nc.compile()
    return nc


def make_in_maps(x, w_qkv, w_proj, b_proj):
    xT = np.ascontiguousarray(
        np.transpose(np.asarray(x, np.float32), (0, 2, 1))
    ).astype(ml_dtypes.bfloat16)
    wq = np.asarray(w_qkv, np.float32).astype(ml_dtypes.bfloat16)
    wp = np.asarray(w_proj, np.float32).astype(ml_dtypes.bfloat16)
    bp = np.ascontiguousarray(np.asarray(b_proj, np.float32))
    return [
        {"xT": np.ascontiguousarray(xT[b]), "w_qkv": wq, "w_proj": wp, "b_proj": bp}
        for b in range(B)
    ]


_BUILT = None


def _get_built():
    global _BUILT
    if _BUILT is None:
        _BUILT = build_nc()
    return _BUILT


def kernel(x, w_qkv, w_proj, b_proj):
    from concourse.bass_utils import run_bass_kernel_spmd

    nc = _get_built()
    in_maps = make_in_maps(x, w_qkv, w_proj, b_proj)
    res = run_bass_kernel_spmd(nc, in_maps, core_ids=list(range(NCORES)))
    return np.stack(
        [
            np.asarray(res.results[b]["out"], np.float32).T
            for b in range(B)
        ]
    )
